# revision 1
# baseline (speedup 1.0000x reference)
"""Trainium2 Bass kernel for nn_DSAGPredictor — fused S/exp/PV pipeline, v3.

S = k^T q = x^T (Wk^T Wq xq): the k projection disappears; raw x tiles
streamed from DRAM are the stationary operand of the S matmuls (2 LDWEIGHTS
per k-tile for S + v2 combined, h-major). One exp per k-tile (strided over
the [128,3,512] staging) with accum_out rowsums. P (bf16) spilled to DRAM.
Three rowsum AllReduces pipelined under the loop; seg0's PV interleaves into
the loop tail (keeps PE duty high for the HAM clock); segs 1-2 PV run
q-chunk-major post-loop with z-outer + output streamed per q-chunk.
"""
import os

from contextlib import ExitStack

import ml_dtypes
import numpy as np

import concourse.bass as bass
import concourse.bacc as bacc
import concourse.tile as tile
from concourse import mybir, bass_utils

N_CORES = 8
CDIM = 256
N_TOK = 9216
NQ = N_TOK // N_CORES   # 1152
QCH = 384
NCH = NQ // QCH         # 3
NKT = N_TOK // 128      # 72
ZDIM = 16
MAXL = 8
C_SHIFT = 96.0
SEGS = [(0, 24), (24, 48), (48, 72)]
RG = 3
PV0_START_KT = 52       # loop slot where seg0 PV bursts begin

f32 = mybir.dt.float32
f32r = mybir.dt.float32r
bf16 = mybir.dt.bfloat16

_CACHE = {}


def _build_nc():
    nc = bacc.Bacc("TRN2", target_bir_lowering=False, debug=False,
                   num_devices=N_CORES)

    xf_d = nc.dram_tensor("xf", [2, 128, N_TOK], f32r, kind="ExternalInput")
    xq_d = nc.dram_tensor("xq", [2, 128, NQ], f32r, kind="ExternalInput")
    wqT_d = nc.dram_tensor("wqT", [2, 128, CDIM], f32r, kind="ExternalInput")
    wk_d = nc.dram_tensor("wk", [2, 128, CDIM], f32r, kind="ExternalInput")
    wv2T_d = nc.dram_tensor("wv2T", [2, 128, CDIM], f32r, kind="ExternalInput")
    wpT_d = nc.dram_tensor("wpT", [2, 128, CDIM], f32r, kind="ExternalInput")
    relT_d = nc.dram_tensor("relT", [2, 128, ZDIM], f32r, kind="ExternalInput")
    wlast_d = nc.dram_tensor("wlast", [1, CDIM], bf16, kind="ExternalInput")
    out_d = nc.dram_tensor("out", [ZDIM, 2, 128, NQ], f32, kind="ExternalOutput")

    seg_len = [b - a for (a, b) in SEGS]
    n_segs = len(SEGS)

    with tile.TileContext(nc) as tc, ExitStack() as ctx:
        const = ctx.enter_context(tc.tile_pool(name="const", bufs=1))
        big = ctx.enter_context(tc.tile_pool(name="big", bufs=1))
        xin = ctx.enter_context(tc.tile_pool(name="xin", bufs=3))
        pout = ctx.enter_context(tc.tile_pool(name="pout", bufs=3))
        pin = ctx.enter_context(tc.tile_pool(name="pin", bufs=3))
        poscp = ctx.enter_context(tc.tile_pool(name="poscp", bufs=1))
        ypool = ctx.enter_context(tc.tile_pool(name="ypool", bufs=2))
        opool = ctx.enter_context(tc.tile_pool(name="opool", bufs=4))
        dram = ctx.enter_context(tc.tile_pool(name="dram", bufs=1, space="DRAM"))

        wv2_r = const.tile([128, 2, CDIM], f32r)
        wp_r = const.tile([128, 2, CDIM], f32r)
        wl_b = const.tile([1, CDIM], bf16)
        negc = const.tile([128, 1], f32)

        xq_r = big.tile([128, 2, NQ], f32r)
        t_s = big.tile([128, 2, NQ], f32r)
        v2seg = [big.tile([128, seg_len[i], CDIM], bf16, name=f"v2s{i}")
                 for i in range(n_segs)]
        pos_s = big.tile([ZDIM, NQ], bf16)
        stats = big.tile([128, NKT], f32)
        stats_tot = big.tile([128, NKT], f32)
        recip = big.tile([128, NKT], f32)

        pspill = [dram.tile([seg_len[i], 128, NQ], bf16, name=f"pspill{i}")
                  for i in range(n_segs)]
        cc_in = [dram.tile([128, seg_len[i]], f32, name=f"cc_in{i}")
                 for i in range(n_segs)]
        cc_out = [dram.tile([128, seg_len[i]], f32, addr_space="Shared",
                            name=f"cc_out{i}")
                  for i in range(n_segs)]

        seg_of = {}
        for i, (a, b) in enumerate(SEGS):
            for kt in range(a, b):
                seg_of[kt] = i
        seg_end = {b - 1: i for i, (a, b) in enumerate(SEGS)}

        def v2ap(kt, hs=slice(None)):
            s = seg_of[kt]
            return v2seg[s][:, kt - SEGS[s][0], hs]

        nc.sync.dma_start(xq_r[:], xq_d[:, :, :].rearrange("h p c -> p h c"))
        nc.sync.dma_start(wv2_r[:], wv2T_d[:, :, :].rearrange("h p c -> p h c"))
        nc.sync.dma_start(wp_r[:], wpT_d[:, :, :].rearrange("h p c -> p h c"))
        nc.sync.dma_start(wl_b[:], wlast_d[:, :])
        nc.vector.memset(negc[:], -C_SHIFT)

        # =========== phase A: T = Wk^T (Wq xq), pos = rel xq ===========
        with tc.tile_pool(name="pA", bufs=1) as pA, \
             tc.tile_pool(name="psA", bufs=4, space="PSUM") as psA:
            wq_r = pA.tile([128, 2, CDIM], f32r)
            wk_r = pA.tile([128, 2, CDIM], f32r)
            rel_r = pA.tile([128, 2, ZDIM], f32r)
            t1_s = pA.tile([128, 2, NQ], f32r)
            nc.sync.dma_start(wq_r[:],
                              wqT_d[:, :, :].rearrange("h p c -> p h c"))
            nc.sync.dma_start(wk_r[:],
                              wk_d[:, :, :].rearrange("h p c -> p h c"))
            nc.sync.dma_start(rel_r[:],
                              relT_d[:, :, :].rearrange("h p c -> p h c"))
            for qc in range(NCH):
                qsl = slice(qc * QCH, (qc + 1) * QCH)
                for h in range(2):
                    hs = slice(h * 128, (h + 1) * 128)
                    ps = psA.tile([128, QCH], f32, tag="mm")
                    nc.tensor.matmul(ps[:], wq_r[:, 0, hs], xq_r[:, 0, qsl],
                                     start=True, stop=False)
                    nc.tensor.matmul(ps[:], wq_r[:, 1, hs], xq_r[:, 1, qsl],
                                     start=False, stop=True)
                    nc.vector.tensor_copy(t1_s[:, h, qsl], ps[:])
                ps_p = psA.tile([ZDIM, QCH], f32, tag="mm")
                nc.tensor.matmul(ps_p[:], rel_r[:, 0, :], xq_r[:, 0, qsl],
                                 start=True, stop=False)
                nc.tensor.matmul(ps_p[:], rel_r[:, 1, :], xq_r[:, 1, qsl],
                                 start=False, stop=True)
                nc.vector.tensor_copy(pos_s[:, qsl], ps_p[:])
                for h in range(2):
                    hs = slice(h * 128, (h + 1) * 128)
                    ps = psA.tile([128, QCH], f32, tag="mm")
                    nc.tensor.matmul(ps[:], wk_r[:, 0, hs], t1_s[:, 0, qsl],
                                     start=True, stop=False)
                    nc.tensor.matmul(ps[:], wk_r[:, 1, hs], t1_s[:, 1, qsl],
                                     start=False, stop=True)
                    nc.vector.tensor_copy(t_s[:, h, qsl], ps[:])

        def _ar_seg(i):
            lo, hi = SEGS[i]
            nc.gpsimd.dma_start(cc_in[i][:], stats[:, lo:hi])
            nc.gpsimd.collective_compute(
                "AllReduce",
                mybir.AluOpType.add,
                replica_groups=[list(range(N_CORES))],
                ins=[cc_in[i][:].opt()],
                outs=[cc_out[i][:].opt()],
            )
            nc.gpsimd.dma_start(stats_tot[:, lo:hi], cc_out[i][:])

        recip_done = set()
        fold_done = set()

        def _fold(si):
            """Reciprocal + fold 1/s into v2 rows for a whole segment (DVE)."""
            if si in fold_done:
                return
            fold_done.add(si)
            lo, hi = SEGS[si]
            nc.vector.reciprocal(recip[:, lo:hi], stats_tot[:, lo:hi])
            for kt in range(lo, hi):
                nc.vector.tensor_scalar_mul(v2ap(kt), v2ap(kt),
                                            recip[:, kt:kt + 1])

        acc = [[None, None] for _ in range(NCH)]

        # =========== main loop: S -> exp -> spill ===========
        def _open_accs():
            for qc in range(NCH):
                qsl = slice(qc * QCH, (qc + 1) * QCH)
                for h in range(2):
                    hs = slice(h * 128, (h + 1) * 128)
                    ac = psAcc.tile([128, QCH], f32, tag="acc",
                                    name=f"acc{qc}_{h}")
                    nc.tensor.matmul(ac[:], wp_r[:, 0, hs], xq_r[:, 0, qsl],
                                     start=True, stop=False)
                    nc.tensor.matmul(ac[:], wp_r[:, 1, hs], xq_r[:, 1, qsl],
                                     start=False, stop=False)
                    acc[qc][h] = ac

        with tc.tile_pool(name="psS", bufs=2, space="PSUM") as psS, \
             tc.tile_pool(name="psV", bufs=2, space="PSUM") as psV:
            for kt in range(NKT):
                seg = seg_of[kt]
                if kt % 2 == 0:
                    xt = xin.tile([128, 2, 256], f32r, tag="xt")
                    sl = slice(kt * 128, kt * 128 + 256)
                    nc.sync.dma_start(
                        xt[:], xf_d[:, :, sl].rearrange("h p c -> p h c"))
                tsl = slice((kt % 2) * 128, (kt % 2) * 128 + 128)
                ps = psS.tile([128, NCH, 512], f32, tag="stg")
                psv = psV.tile([128, CDIM], f32, tag="v2s")
                for h in range(2):
                    st, sp = h == 0, h == 1
                    for qc in range(NCH):
                        nc.tensor.matmul(ps[:, qc, 0:QCH], xt[:, h, tsl],
                                         t_s[:, h, qc * QCH:(qc + 1) * QCH],
                                         start=st, stop=sp)
                    nc.tensor.matmul(psv[:], xt[:, h, tsl], wv2_r[:, h, :],
                                     start=st, stop=sp)
                pt = pout.tile([128, NQ], bf16, tag="pt")
                pt3 = pt[:].rearrange("p (c q) -> p c q", c=NCH)
                nc.scalar.activation(
                    pt3[:, :, :], ps[:, :, 0:QCH],
                    mybir.ActivationFunctionType.Exp,
                    bias=negc[:], scale=1.0,
                    accum_out=stats[:, kt:kt + 1])
                nc.vector.tensor_copy(v2ap(kt), psv[:])
                nc.sync.dma_start(pspill[seg][kt - SEGS[seg][0]], pt[:])
                if kt in seg_end:
                    _ar_seg(seg_end[kt])

        psAcc = ctx.enter_context(tc.tile_pool(name="psAcc", bufs=6,
                                               space="PSUM"))
        _open_accs()

        def _pv_qc(qc, si, stop_seg):
            """Post-loop PV for one (q-chunk, segment)."""
            _fold(si)
            lo, hi = SEGS[si]
            qsl = slice(qc * QCH, (qc + 1) * QCH)
            for kt0 in range(lo, hi, RG):
                g = min(RG, hi - kt0)
                stg = pin.tile([128, RG, QCH], bf16, tag="pinq",
                               name=f"pinq{qc}_{kt0}")
                nc.sync.dma_start(
                    stg[:, 0:g, :],
                    pspill[si][kt0 - lo:kt0 - lo + g, :, qsl]
                    .rearrange("g p c -> p g c"))
                for j in range(g):
                    kt = kt0 + j
                    last = stop_seg and (kt == hi - 1)
                    for h in range(2):
                        hs = slice(h * 128, (h + 1) * 128)
                        nc.tensor.matmul(acc[qc][h][:], v2ap(kt, hs),
                                         stg[:, j, :],
                                         start=False, stop=last)

        def _tail_qc(qc, psO):
            qsl = slice(qc * QCH, (qc + 1) * QCH)
            posc = poscp.tile([1, ZDIM, QCH], bf16, tag="posc")
            nc.sync.dma_start(posc[0:1, :, :], pos_s[:, qsl])
            ys = []
            for oh in range(2):
                y = ypool.tile([128, QCH], f32, tag="y")
                nc.vector.tensor_copy(y[:], acc[qc][oh][:])
                ys.append(y)
            for z in range(ZDIM):
                for oh in range(2):
                    ps_o = psO.tile([128, QCH], f32, tag="zmm")
                    nc.tensor.matmul(
                        ps_o[:], wl_b[0:1, oh * 128:(oh + 1) * 128],
                        posc[0:1, z, :], start=True, stop=True)
                    ot = opool.tile([128, QCH], f32, tag="ot")
                    nc.vector.tensor_add(ot[:], ps_o[:], ys[oh][:])
                    nc.scalar.dma_start(out_d[z, oh, :, qsl], ot[:])

        # =========== post-loop: seg1 PV, then seg2 + tail per q-chunk ======
        with tc.tile_pool(name="psO", bufs=2, space="PSUM") as psO:
            for qc in range(NCH):
                _pv_qc(qc, 0, stop_seg=False)
                _pv_qc(qc, 1, stop_seg=False)
            for qc in range(NCH):
                _pv_qc(qc, 2, stop_seg=True)
                _tail_qc(qc, psO)

    nc.compile()
    return nc


def _get_nc():
    if "nc" not in _CACHE:
        _CACHE["nc"] = _build_nc()
    return _CACHE["nc"]


def _prep_in_maps(x, Wq, Wk, Wv, embd, Wproj, dist, isWithin):
    x = np.asarray(x, np.float32)
    Wq = np.asarray(Wq, np.float32)
    Wk = np.asarray(Wk, np.float32)
    Wv = np.asarray(Wv, np.float32)
    embd = np.asarray(embd, np.float32)
    Wproj = np.asarray(Wproj, np.float32)

    xf = np.ascontiguousarray(x.reshape(CDIM, N_TOK))
    WprojC = Wproj[:, :CDIM]
    wlast = np.ascontiguousarray(Wproj[:, CDIM]).reshape(1, CDIM)
    Wv2 = WprojC @ Wv
    dist = np.asarray(dist).astype(np.int64)
    isWithin = np.asarray(isWithin).astype(np.int64)
    rel = embd[isWithin, dist + MAXL]

    def split2(a):
        return np.ascontiguousarray(a.reshape(2, 128, -1), dtype=np.float32)

    common = {
        "xf": split2(xf),
        "wqT": split2(Wq.T),
        "wk": split2(Wk),
        "wv2T": split2(Wv2.T),
        "wpT": split2(WprojC.T),
        "relT": split2(rel.T),
        "wlast": wlast.astype(ml_dtypes.bfloat16),
    }
    in_maps = []
    for c in range(N_CORES):
        m = dict(common)
        m["xq"] = split2(np.ascontiguousarray(xf[:, c * NQ:(c + 1) * NQ]))
        in_maps.append(m)
    return in_maps


def run(inputs, trace=False, tmpdir=None):
    nc = _get_nc()
    in_maps = _prep_in_maps(**inputs)
    res = bass_utils.run_bass_kernel_spmd(
        nc, in_maps, core_ids=list(range(N_CORES)), trace=trace, tmpdir=tmpdir,
    )
    parts = [res.results[c]["out"].reshape(ZDIM, CDIM, NQ)
             for c in range(N_CORES)]
    full = np.concatenate(parts, axis=2).reshape(ZDIM, CDIM, 96, 96)
    return np.ascontiguousarray(full.astype(np.float32)), res


def kernel(**inputs) -> np.ndarray:
    out, _ = run(inputs, trace=bool(int(os.environ.get("KERNEL_TRACE", "0"))))
    return out



# revision 20
# speedup vs baseline: 1.0276x; 1.0276x over previous
"""Trainium2 Bass kernel for nn_DSAGPredictor — v4.

Key changes vs v3 (369us):
- P (exp(S-96), bf16) kept resident in SBUF for 52/72 k-tiles; only the
  first 20 spill to DRAM (42.5MB -> 11.8MB of P traffic).
- All per-k-tile stationaries are bf16 (x, Wv2 host-cast): their ~97ns
  self-LDWEIGHTS hides under the >=160ns matmuls, vs 190ns+ exposed for
  f32r. No ldweights reuse - the Tile scheduler may interleave matmuls,
  which clobbers reused weights (found the hard way).
- 1/D folds into v2 on DVE post-loop, per-kt granularity so PV starts
  immediately.
- PV runs kt-major straight from SBUF P into 6 persistent PSUM accs.
- Tail adds the residual Y2 into PSUM via identity matmuls (f32r) so
  the evacuation is a pure cast copy, split DVE/ACT; output stored bf16
  in a [2,128,qc,z,384] layout for 12KB/partition DMA lines.
- ~8us of junk warm-up matmuls under the initial DMAs keep the HAM
  clock gate open before phase A.
"""
import os

from contextlib import ExitStack

import ml_dtypes
import numpy as np

import concourse.bass as bass
import concourse.bacc as bacc
import concourse.tile as tile
from concourse import mybir, bass_utils

N_CORES = 8
CDIM = 256
N_TOK = 9216
NQ = N_TOK // N_CORES   # 1152
QCH = 384
NCH = NQ // QCH         # 3
NKT = N_TOK // 128      # 72
ZDIM = 16
MAXL = 8
C_SHIFT = 96.0
SEGS = [(0, 24), (24, 48), (48, 72)]
SPILL_KT = 21           # k-tiles 0..20 spill to DRAM; the rest stay in SBUF
WARMUP_MM = 20

f32 = mybir.dt.float32
f32r = mybir.dt.float32r
bf16 = mybir.dt.bfloat16

_CACHE = {}


def _build_nc():
    nc = bacc.Bacc("TRN2", target_bir_lowering=False, debug=False,
                   num_devices=N_CORES)

    xf_d = nc.dram_tensor("xf", [2, 128, N_TOK], bf16, kind="ExternalInput")
    xq_d = nc.dram_tensor("xq", [2, 128, NQ], f32r, kind="ExternalInput")
    wqT_d = nc.dram_tensor("wqT", [2, 128, CDIM], f32r, kind="ExternalInput")
    wk_d = nc.dram_tensor("wk", [2, 128, CDIM], f32r, kind="ExternalInput")
    wv2T_d = nc.dram_tensor("wv2T", [2, 128, CDIM], bf16, kind="ExternalInput")
    wpT_d = nc.dram_tensor("wpT", [2, 128, CDIM], f32r, kind="ExternalInput")
    relT_d = nc.dram_tensor("relT", [2, 128, ZDIM], f32r, kind="ExternalInput")
    wlast_d = nc.dram_tensor("wlast", [1, CDIM], bf16, kind="ExternalInput")
    id_d = nc.dram_tensor("ident", [128, 128], bf16, kind="ExternalInput")
    out_d = nc.dram_tensor("out", [2, 128, NCH, ZDIM, QCH], bf16,
                           kind="ExternalOutput")
    DEBUG = bool(int(os.environ.get("KERNEL_DEBUG", "0")))
    if DEBUG:
        dbg_stats = nc.dram_tensor("dbg_stats", [128, NKT], f32,
                                   kind="ExternalOutput")
        dbg_stot = nc.dram_tensor("dbg_stot", [128, NKT], f32,
                                  kind="ExternalOutput")
        dbg_v2 = nc.dram_tensor("dbg_v2", [128, NKT, CDIM], bf16,
                                kind="ExternalOutput")
        dbg_y = nc.dram_tensor("dbg_y", [128, 6, QCH], bf16,
                               kind="ExternalOutput")
        dbg_pos = nc.dram_tensor("dbg_pos", [ZDIM, NQ], bf16,
                                 kind="ExternalOutput")
        dbg_t = nc.dram_tensor("dbg_t", [128, 2, NQ], bf16,
                               kind="ExternalOutput")
        dbg_p = nc.dram_tensor("dbg_p", [128, 2, NQ], bf16,
                               kind="ExternalOutput")

    n_segs = len(SEGS)
    seg_end = {b - 1: i for i, (a, b) in enumerate(SEGS)}

    with tile.TileContext(nc) as tc, ExitStack() as ctx:
        const = ctx.enter_context(tc.tile_pool(name="const", bufs=1))
        big = ctx.enter_context(tc.tile_pool(name="big", bufs=1))
        xin = ctx.enter_context(tc.tile_pool(name="xin", bufs=3))
        pout = ctx.enter_context(tc.tile_pool(name="pout", bufs=2))
        pin = ctx.enter_context(tc.tile_pool(name="pin", bufs=3))
        dram = ctx.enter_context(tc.tile_pool(name="dram", bufs=1, space="DRAM"))

        wv2_r = const.tile([128, 2, CDIM], bf16)
        wp_r = const.tile([128, 2, CDIM], f32r)
        wl_b = const.tile([1, CDIM], bf16)
        ident_r = const.tile([128, 128], bf16)
        rel_r = const.tile([128, 2, ZDIM], f32r)
        negc = const.tile([128, 1], f32)
        wz = const.tile([128, 128], bf16)
        mz = const.tile([128, 512], bf16)

        xq_r = big.tile([128, 2, NQ], f32r)
        t_s = big.tile([128, 2, NQ], bf16)
        v2 = big.tile([128, NKT, CDIM], bf16)
        pos_s = big.tile([ZDIM, NQ], bf16)
        stats = big.tile([128, NKT], f32)
        stats_tot = big.tile([128, NKT], f32)
        recip = big.tile([128, NKT], f32)

        pspill = dram.tile([SPILL_KT, 128, NQ], bf16)
        cc_in = [dram.tile([128, b - a], f32, name=f"cc_in{i}")
                 for i, (a, b) in enumerate(SEGS)]
        cc_out = [dram.tile([128, b - a], f32, addr_space="Shared",
                            name=f"cc_out{i}")
                  for i, (a, b) in enumerate(SEGS)]

        # input DMAs (phase-A weights first) + warm-up matmuls under them
        nc.sync.dma_start(xq_r[:], xq_d[:, :, :].rearrange("h p c -> p h c"))
        nc.vector.memset(negc[:], -C_SHIFT)
        nc.vector.memset(wz[:], 0.0)
        nc.vector.memset(mz[:], 0.0)

        with tc.tile_pool(name="pA", bufs=1) as pA, \
             tc.tile_pool(name="psA", bufs=1, space="PSUM") as psA:
            wq_r = pA.tile([128, 2, CDIM], f32r)
            wk_r = pA.tile([128, 2, CDIM], f32r)
            t1_s = pA.tile([128, 2, NQ], f32r)
            nc.sync.dma_start(wq_r[:],
                              wqT_d[:, :, :].rearrange("h p c -> p h c"))
            nc.sync.dma_start(wk_r[:],
                              wk_d[:, :, :].rearrange("h p c -> p h c"))
            nc.sync.dma_start(rel_r[:],
                              relT_d[:, :, :].rearrange("h p c -> p h c"))
            nc.sync.dma_start(wv2_r[:],
                              wv2T_d[:, :, :].rearrange("h p c -> p h c"))
            nc.sync.dma_start(wp_r[:],
                              wpT_d[:, :, :].rearrange("h p c -> p h c"))
            nc.sync.dma_start(wl_b[:], wlast_d[:, :])
            nc.sync.dma_start(ident_r[:], id_d[:, :])

            ps_w = psA.tile([128, 512], f32, tag="Ap", bufs=2, name="ps_w")
            for i in range(WARMUP_MM):
                nc.tensor.matmul(ps_w[:], wz[:], mz[:], start=True, stop=True)

            # ======= phase A: T = Wk^T (Wq xq), pos = rel xq =======
            def _round(w_r, src, dst):
                ps6 = psA.tile([128, 6, 512], f32, tag="A6", name="ps6")
                for hs in range(2):
                    for hc in range(2):
                        for qc in range(NCH):
                            nc.tensor.matmul(
                                ps6[:, hs * NCH + qc, 0:QCH],
                                w_r[:, hc, hs * 128:(hs + 1) * 128],
                                src[:, hc, qc * QCH:(qc + 1) * QCH],
                                start=(hc == 0), stop=(hc == 1))
                for hs in range(2):
                    for qc in range(NCH):
                        nc.vector.tensor_copy(
                            dst[:, hs, qc * QCH:(qc + 1) * QCH],
                            ps6[:, hs * NCH + qc, 0:QCH])

            _round(wq_r, xq_r, t1_s)
            for qc in range(NCH):
                ps_p = psA.tile([ZDIM, 512], f32, tag="Ap", bufs=2,
                                name="ps_p")
                for hc in range(2):
                    nc.tensor.matmul(
                        ps_p[:, 0:QCH], rel_r[:, hc, :],
                        xq_r[:, hc, qc * QCH:(qc + 1) * QCH],
                        start=(hc == 0), stop=(hc == 1))
                nc.vector.tensor_copy(pos_s[:, qc * QCH:(qc + 1) * QCH],
                                      ps_p[:, 0:QCH])
            _round(wk_r, t1_s, t_s)

        def _ar_seg(i):
            lo, hi = SEGS[i]
            nc.gpsimd.dma_start(cc_in[i][:], stats[:, lo:hi])
            nc.gpsimd.collective_compute(
                "AllReduce",
                mybir.AluOpType.add,
                replica_groups=[list(range(N_CORES))],
                ins=[cc_in[i][:].opt()],
                outs=[cc_out[i][:].opt()],
            )
            nc.gpsimd.dma_start(stats_tot[:, lo:hi], cc_out[i][:])

        with tc.tile_pool(name="pbig", bufs=1) as pbig:
            p_sb = pbig.tile([128, NKT - SPILL_KT, NQ], bf16)

            # =========== main loop: S -> exp -> P (SBUF or spill) ========
            with tc.tile_pool(name="psS", bufs=2, space="PSUM") as psS, \
                 tc.tile_pool(name="psV", bufs=2, space="PSUM") as psV:
                for kt in range(NKT):
                    if kt % 2 == 0:
                        xt = xin.tile([128, 2, 256], bf16, tag="xt",
                                      name="xt")
                        sl = slice(kt * 128, kt * 128 + 256)
                        nc.sync.dma_start(
                            xt[:], xf_d[:, :, sl].rearrange("h p c -> p h c"))
                    tsl = slice((kt % 2) * 128, (kt % 2) * 128 + 128)
                    ps = psS.tile([128, NCH, 512], f32, tag="stg", name="ps")
                    psv = psV.tile([128, CDIM], f32, tag="v2s", name="psv")
                    for h in range(2):
                        st, sp = h == 0, h == 1
                        for qc in range(NCH):
                            nc.tensor.matmul(
                                ps[:, qc, 0:QCH], xt[:, h, tsl],
                                t_s[:, h, qc * QCH:(qc + 1) * QCH],
                                start=st, stop=sp)
                        nc.tensor.matmul(psv[:], xt[:, h, tsl],
                                              wv2_r[:, h, :], start=st,
                                              stop=sp)
                    if kt < SPILL_KT:
                        pt = pout.tile([128, NQ], bf16, tag="pt", name="pt")
                    else:
                        pt = p_sb[:, kt - SPILL_KT, :]
                    pt3 = pt.rearrange("p (c q) -> p c q", c=NCH)
                    nc.scalar.activation(
                        pt3[:, :, :], ps[:, :, 0:QCH],
                        mybir.ActivationFunctionType.Exp,
                        bias=negc[:], scale=1.0,
                        accum_out=stats[:, kt:kt + 1])
                    nc.vector.tensor_copy(v2[:, kt, :], psv[:])
                    if kt < SPILL_KT:
                        nc.gpsimd.dma_start(pspill[kt], pt[:, :])
                    if kt in seg_end:
                        _ar_seg(seg_end[kt])

            # recips + 1/D folds into v2, all on DVE (idle post-loop)
            for i in range(n_segs):
                lo, hi = SEGS[i]
                nc.vector.reciprocal(recip[:, lo:hi], stats_tot[:, lo:hi])
            for kt in range(NKT):
                nc.vector.tensor_scalar_mul(v2[:, kt, :], v2[:, kt, :],
                                            recip[:, kt:kt + 1])

            # ====== PV: kt-major from SBUF/reload into 6 PSUM accs =======
            with tc.tile_pool(name="psAcc", bufs=1, space="PSUM") as psAcc, \
                 tc.tile_pool(name="ptail", bufs=1) as ptail:
                acc = psAcc.tile([128, 6, 512], f32)
                for h in range(2):
                    for hc in range(2):
                        for qc in range(NCH):
                            nc.tensor.matmul(
                                acc[:, qc * 2 + h, 0:QCH],
                                wp_r[:, hc, h * 128:(h + 1) * 128],
                                xq_r[:, hc, qc * QCH:(qc + 1) * QCH],
                                start=(hc == 0), stop=False)

                for kt in range(NKT - 6):
                    if kt < SPILL_KT:
                        prow = pin.tile([128, NQ], bf16, tag="pin",
                                        name="prow")
                        nc.sync.dma_start(prow[:, :], pspill[kt])
                    else:
                        prow = p_sb[:, kt - SPILL_KT, :]
                    for h in range(2):
                        for qc in range(NCH):
                            nc.tensor.matmul(
                                acc[:, qc * 2 + h, 0:QCH],
                                v2[:, kt, h * 128:(h + 1) * 128],
                                prow[:, qc * QCH:(qc + 1) * QCH],
                                start=False, stop=False)

                # last 6 k-tiles acc-major so y-evacs overlap remaining PV
                y = ptail.tile([128, 6, QCH], bf16)
                for qc in range(NCH):
                    for h in range(2):
                        for kt in range(NKT - 6, NKT):
                            nc.tensor.matmul(
                                acc[:, qc * 2 + h, 0:QCH],
                                v2[:, kt, h * 128:(h + 1) * 128],
                                p_sb[:, kt - SPILL_KT,
                                     qc * QCH:(qc + 1) * QCH],
                                start=False, stop=(kt == NKT - 1))
                        nc.vector.tensor_copy(y[:, qc * 2 + h, :],
                                              acc[:, qc * 2 + h, 0:QCH])
                if DEBUG:
                    nc.sync.dma_start(dbg_p[:, 0, :], p_sb[:, 0, :])
                    nc.sync.dma_start(dbg_p[:, 1, :], p_sb[:, 1, :])

        # =========== tail: out[z] = Y2 + wlast (x) pos[z] ================
        with tc.tile_pool(name="psO", bufs=2, space="PSUM") as psO, \
             tc.tile_pool(name="potail", bufs=2) as potail, \
             tc.tile_pool(name="poscp", bufs=3) as poscp:
            poscs = []
            for qc in range(NCH):
                posc = poscp.tile([1, ZDIM, QCH], bf16, tag="posc",
                                  name="posc")
                nc.sync.dma_start(posc[0:1, :, :],
                                  pos_s[:, qc * QCH:(qc + 1) * QCH])
                poscs.append(posc)
            for qc in range(NCH):
                posc = poscs[qc]
                ost = potail.tile([128, 2, ZDIM, QCH], bf16, tag="ost",
                                  name="ost")
                for h in range(2):
                    for zg in range(ZDIM // 4):
                        ps4 = psO.tile([128, 4, 512], f32, tag="zg",
                                       name="ps4")
                        for j in range(4):
                            nc.tensor.matmul(
                                ps4[:, j, 0:QCH],
                                wl_b[0:1, h * 128:(h + 1) * 128],
                                posc[0:1, zg * 4 + j, :],
                                start=True, stop=False)
                        for j in range(4):
                            nc.tensor.matmul(
                                ps4[:, j, 0:QCH], ident_r[:, :],
                                y[:, qc * 2 + h, :],
                                start=False, stop=True)
                        dst = ost[:, h, zg * 4:(zg + 1) * 4, :]
                        if zg % 2 == 0:
                            nc.vector.tensor_copy(dst, ps4[:, :, 0:QCH])
                        else:
                            nc.scalar.copy(dst, ps4[:, :, 0:QCH])
                for h in range(2):
                    for zh in range(2):
                        zs = slice(zh * 8, (zh + 1) * 8)
                        nc.gpsimd.dma_start(
                            out_d[h, :, qc, zs, :],
                            ost[:, h, zs, :])
            if DEBUG:
                nc.sync.dma_start(dbg_stats[:, :], stats[:, :])
                nc.sync.dma_start(dbg_stot[:, :], stats_tot[:, :])
                nc.sync.dma_start(dbg_v2[:, :, :], v2[:, :, :])
                nc.sync.dma_start(dbg_y[:, :, :], y[:, :, :])
                nc.sync.dma_start(dbg_pos[:, :], pos_s[:, :])
                nc.sync.dma_start(dbg_t[:, :, :], t_s[:, :, :])

    nc.compile()
    return nc


def _get_nc():
    if "nc" not in _CACHE:
        _CACHE["nc"] = _build_nc()
    return _CACHE["nc"]


def _prep_in_maps(x, Wq, Wk, Wv, embd, Wproj, dist, isWithin):
    x = np.asarray(x, np.float32)
    Wq = np.asarray(Wq, np.float32)
    Wk = np.asarray(Wk, np.float32)
    Wv = np.asarray(Wv, np.float32)
    embd = np.asarray(embd, np.float32)
    Wproj = np.asarray(Wproj, np.float32)

    xf = np.ascontiguousarray(x.reshape(CDIM, N_TOK))
    WprojC = Wproj[:, :CDIM]
    wlast = np.ascontiguousarray(Wproj[:, CDIM]).reshape(1, CDIM)
    Wv2 = WprojC @ Wv
    dist = np.asarray(dist).astype(np.int64)
    isWithin = np.asarray(isWithin).astype(np.int64)
    rel = embd[isWithin, dist + MAXL]

    def split2(a):
        return np.ascontiguousarray(a.reshape(2, 128, -1), dtype=np.float32)

    def split2b(a):
        return split2(a).astype(ml_dtypes.bfloat16)

    common = {
        "xf": split2b(xf),
        "wqT": split2(Wq.T),
        "wk": split2(Wk),
        "wv2T": split2b(Wv2.T),
        "wpT": split2(WprojC.T),
        "relT": split2(rel.T),
        "wlast": wlast.astype(ml_dtypes.bfloat16),
        "ident": np.eye(128, dtype=np.float32).astype(ml_dtypes.bfloat16),
    }
    in_maps = []
    for c in range(N_CORES):
        m = dict(common)
        m["xq"] = split2(np.ascontiguousarray(xf[:, c * NQ:(c + 1) * NQ]))
        in_maps.append(m)
    return in_maps


def run(inputs, trace=False, tmpdir=None):
    nc = _get_nc()
    in_maps = _prep_in_maps(**inputs)
    res = bass_utils.run_bass_kernel_spmd(
        nc, in_maps, core_ids=list(range(N_CORES)), trace=trace, tmpdir=tmpdir,
    )
    parts = []
    for c in range(N_CORES):
        a = np.asarray(res.results[c]["out"]).astype(np.float32)
        # [2, 128, NCH, ZDIM, QCH] -> [ZDIM, 256, NQ]
        a = a.transpose(3, 0, 1, 2, 4).reshape(ZDIM, CDIM, NQ)
        parts.append(a)
    full = np.concatenate(parts, axis=2).reshape(ZDIM, CDIM, 96, 96)
    return np.ascontiguousarray(full), res


def kernel(**inputs) -> np.ndarray:
    out, _ = run(inputs, trace=bool(int(os.environ.get("KERNEL_TRACE", "0"))))
    return out
